# revision 54
# baseline (speedup 1.0000x reference)
"""Distributed Bass kernel for nn_Attention (B=2, T=2048, D=1024, H=16) on 8 TRN2 cores.

Sharding: core c -> (batch b = c//4, head-group g = c%4, heads 4g..4g+3).
QKV tensor-parallel over heads, out-proj row-parallel + ReduceScatter(4-rank groups).
"""

import functools
import numpy as np
from contextlib import ExitStack

B, T, D, H, HD = 2, 2048, 1024, 16, 64
EPS = 1e-4
NCORES, GROUP = 8, 4
HL = H // GROUP          # heads per core = 4
DL = HL * HD             # local feature cols = 256
NTT = T // 128           # 16 token tiles
NDT = D // 128           # 8 d tiles
NWT = (3 * DL) // 128    # 6 w_qkv row tiles
NIKB = T // 1024         # 2 query kilo-blocks


def _build_bass():
    import concourse.bass as bass
    import concourse.tile as tile
    from concourse import bacc, mybir

    f32 = mybir.dt.float32
    f32r = mybir.dt.float32r
    bf16 = mybir.dt.bfloat16
    AX = mybir.AxisListType
    OP = mybir.AluOpType
    AF = mybir.ActivationFunctionType

    nc = bacc.Bacc("TRN2", target_bir_lowering=False, debug=False, num_devices=NCORES)

    xT_ext = nc.dram_tensor("xT", [D, T], bf16, kind="ExternalInput").ap()
    wqkv_ext = nc.dram_tensor("wqkv", [3 * DL, D], bf16, kind="ExternalInput").ap()
    woutT_ext = nc.dram_tensor("woutT", [DL, D], bf16, kind="ExternalInput").ap()
    wout_ext = nc.dram_tensor("wout", [D, D], bf16, kind="ExternalInput").ap()
    out_ext = nc.dram_tensor("out", [DL, T], f32, kind="ExternalOutput").ap()

    import ml_dtypes
    ident_np = np.eye(128, dtype=ml_dtypes.bfloat16)
    ones_np = np.ones((1, 64), dtype=np.float32)

    with tile.TileContext(nc) as tc, ExitStack() as ctx:
        # ---------------- persistent pools ----------------
        pers = ctx.enter_context(tc.tile_pool(name="pers", bufs=1))
        dram = ctx.enter_context(tc.tile_pool(name="dram", bufs=1, space="DRAM"))

        id_sb = pers.tile([128, 128], bf16)
        ones_sb = pers.tile([1, 64], f32r)
        ident_dram = nc.inline_tensor(ident_np, name="ident_c")
        ones_dram = nc.inline_tensor(ones_np, name="ones_c")
        nc.sync.dma_start(id_sb[:], ident_dram.ap())
        nc.gpsimd.dma_start(ones_sb[:], ones_dram.ap())

        xT_sb = pers.tile([128, NDT, T], bf16)
        whT_sb = pers.tile([128, NDT, 3 * DL], bf16)
        WT_sb = pers.tile([128, 2, D], bf16)
        qT_sb = [pers.tile([128, T], bf16, name=f"qT{rb}") for rb in range(2)]
        kT_sb = [pers.tile([128, T], bf16, name=f"kT{rb}") for rb in range(2)]
        # per-head duplicated layouts (head h at partitions 0-63 AND 64-127)
        # for 2x row-tiled score matmuls on j-parity
        qTd = [pers.tile([128, T], bf16, name=f"qTd{h}") for h in range(HL)]
        kTd = [pers.tile([128, T], bf16, name=f"kTd{h}") for h in range(HL)]
        v_sb = pers.tile([128, NTT, HL * 128], bf16)
        b_all = pers.tile([128, NTT, HL], f32)
        s_out = pers.tile([128, NDT], f32)
        aoT_sb = [pers.tile([128, T], bf16, name=f"aoT{rb}") for rb in range(2)]

        NRS = 4  # ReduceScatter chunks over tokens
        qnat = dram.tile([T, DL], bf16)
        knat = dram.tile([T, DL], bf16)
        rs_in = [dram.tile([D, T // NRS], bf16, name=f"rs_in{k}")
                 for k in range(NRS)]
        rs_out = [dram.tile([DL, T // NRS], bf16, name=f"rs_out{k}")
                  for k in range(NRS)]

        # ---------------- input DMAs (ordered: wqkv, x, then late weights) ----
        with tc.tile_pool(name="wphase", bufs=1) as wp, \
             tc.tile_pool(name="wpsum", bufs=2, space="PSUM") as wps:
            w_sb = wp.tile([128, NWT, D], bf16)
            wo_sb = wp.tile([128, NDT, D], bf16)
            nc.sync.dma_start(w_sb[:], wqkv_ext.rearrange("(n p) d -> p n d", p=128))
            for half in range(2):
                nc.gpsimd.dma_start(
                    xT_sb[:, 4 * half : 4 * (half + 1), :],
                    xT_ext.rearrange("(n p) t -> p n t", p=128)[:, 4 * half : 4 * (half + 1), :],
                )
            nc.sync.dma_start(WT_sb[:], woutT_ext.rearrange("(n p) d -> p n d", p=128))
            nc.sync.dma_start(wo_sb[:], wout_ext.rearrange("(n p) d -> p n d", p=128))
            nc.vector.memset(v_sb[:], 0.0)
            nc.vector.memset(
                v_sb[:].rearrange("p t (h c) -> p t h c", c=128)[:, :, :, 64:65],
                1.0)

            # PE warm-up: dense dependency-free matmuls while x streams in
            for wu in range(40):
                wt = wps.tile([128, 512], f32, name="wt", tag="warm")
                nc.tensor.matmul(wt[:], w_sb[:, 0, 0:128], w_sb[:, 0, 0:512],
                                 start=True, stop=True)

            # ---------------- phase W: normalize weights ----------------
            sq_scr = wp.tile([128, D], bf16)
            n2w = wp.tile([128, NWT], f32)
            n2o = wp.tile([128, NDT], f32)
            for n in range(NWT):
                nc.scalar.activation(sq_scr[:], w_sb[:, n, :], AF.Square,
                                     accum_out=n2w[:, n : n + 1])
            # s = 1/(norm + 32*eps)  [w_hat = w / (eps + norm/32) / 32]
            s_w = wp.tile([128, NWT], f32)
            nc.scalar.sqrt(n2w[:], n2w[:])
            nc.vector.tensor_scalar_add(n2w[:], n2w[:], 32.0 * EPS)
            nc.vector.reciprocal(s_w[:], n2w[:])

            what = wp.tile([128, NWT, D], bf16)
            for n in range(NWT):
                nc.vector.tensor_scalar_mul(what[:, n, :], w_sb[:, n, :],
                                            s_w[:, n : n + 1])
            # transpose w_hat [768, 1024] -> whT [1024, 768] via PE
            for n in range(NWT):
                for dt_ in range(NDT):
                    tp = wps.tile([128, 128], bf16)
                    nc.tensor.transpose(
                        tp[:], what[:, n, 128 * dt_ : 128 * (dt_ + 1)], id_sb[:])
                    nc.vector.tensor_copy(
                        whT_sb[:, dt_, 128 * n : 128 * (n + 1)], tp[:])

            # w_out row norms (needed only at out-proj eviction)
            for n in range(NDT):
                nc.scalar.activation(sq_scr[:], wo_sb[:, n, :], AF.Square,
                                     accum_out=n2o[:, n : n + 1])
            nc.scalar.sqrt(n2o[:], n2o[:])
            nc.vector.tensor_scalar_add(n2o[:], n2o[:], 32.0 * EPS)
            nc.vector.reciprocal(s_out[:], n2o[:])

        # ---------------- phase QKV (+ attention pass A on token half 0) ----
        with tc.tile_pool(name="qkvps", bufs=2, space="PSUM") as qps, \
             tc.tile_pool(name="qkvsb", bufs=3) as qsb:
            for tt in range(NTT):
                ps = qps.tile([128, 3 * DL], f32)
                for dt_ in range(NDT):
                    lhsT = xT_sb[:, dt_, 128 * tt : 128 * (tt + 1)]
                    nc.tensor.matmul(ps[:, 0:512], lhsT, whT_sb[:, dt_, 0:512],
                                     start=(dt_ == 0), stop=(dt_ == NDT - 1))
                    nc.tensor.matmul(ps[:, 512:768], lhsT, whT_sb[:, dt_, 512:768],
                                     start=(dt_ == 0), stop=(dt_ == NDT - 1))
                # evict q+k raw to sbuf, then norms from the sbuf copy
                qk_raw = qsb.tile([128, 2 * DL], bf16)
                nc.scalar.copy(qk_raw[:], ps[:, 0 : 2 * DL])
                nc.scalar.activation(
                    v_sb[:, tt, :].rearrange("p (h c) -> p h c", c=128)[:, :, 0:HD],
                    ps[:, 2 * DL : 3 * DL].rearrange("p (h c) -> p h c", c=HD),
                    AF.Copy)
                sq = qsb.tile([128, 2 * DL], f32)
                nc.vector.tensor_tensor(sq[:], qk_raw[:], qk_raw[:], op=OP.mult)
                n2 = qsb.tile([128, 2 * HL], f32)
                nc.vector.reduce_sum(
                    n2[:], sq[:].rearrange("p (h c) -> p h c", c=HD), axis=AX.X)
                nc.scalar.sqrt(n2[:], n2[:])
                nc.vector.tensor_scalar_add(n2[:, 0:HL], n2[:, 0:HL], 8.0 * EPS)
                # k scale includes the 1/2 for the duplicated-K scores matmul
                nc.vector.tensor_scalar(n2[:, HL : 2 * HL], n2[:, HL : 2 * HL],
                                        2.0, 16.0 * EPS, op0=OP.mult, op1=OP.add)
                a_q = qsb.tile([128, HL], f32)
                nc.vector.reciprocal(a_q[:], n2[:, 0:HL])
                nc.vector.reciprocal(b_all[:, tt, :], n2[:, HL : 2 * HL])
                # scale q by 8*a (per head), k stays raw
                qst = qsb.tile([128, DL], bf16)
                for h in range(HL):
                    nc.vector.tensor_scalar(
                        qst[:, HD * h : HD * (h + 1)],
                        qk_raw[:, HD * h : HD * (h + 1)],
                        a_q[:, h : h + 1], 8.0, op0=OP.mult, op1=OP.mult)
                nc.sync.dma_start(qnat[128 * tt : 128 * (tt + 1), :], qst[:])
                nc.sync.dma_start(knat[128 * tt : 128 * (tt + 1), :],
                                  qk_raw[:, DL : 2 * DL])
                if tt % 8 == 7:
                    # transpose+duplicate the finished token half right away
                    th = tt // 8
                    tsl = slice(1024 * th, 1024 * (th + 1))
                    for rb in range(2):
                        nc.sync.dma_start_transpose(
                            qT_sb[rb][:, tsl], qnat[tsl, 128 * rb : 128 * (rb + 1)])
                        nc.sync.dma_start_transpose(
                            kT_sb[rb][:, tsl], knat[tsl, 128 * rb : 128 * (rb + 1)])
                    for h in range(HL):
                        rb, hh = h // 2, h % 2
                        for half in range(2):
                            nc.vector.tensor_copy(
                                qTd[h][64 * half : 64 * (half + 1), tsl],
                                qT_sb[rb][64 * hh : 64 * (hh + 1), tsl])
                            nc.vector.tensor_copy(
                                kTd[h][64 * half : 64 * (half + 1), tsl],
                                kT_sb[rb][64 * hh : 64 * (hh + 1), tsl])

        # ---------------- ATTN (ikb-outer) + overlapped OUTPROJ/RS ----------
        with tc.tile_pool(name="scps", bufs=2, space="PSUM") as scps, \
             tc.tile_pool(name="atps", bufs=2, space="PSUM") as atps, \
             tc.tile_pool(name="exsb", bufs=4) as exsb, \
             tc.tile_pool(name="rssb", bufs=2) as rssb, \
             tc.tile_pool(name="ysb", bufs=2) as ysb:

            def rs_dance(po, h, tok0, width):
                """divide outT rows 0..63 by rowsum row 64, write into aoT."""
                rb, hh = h // 2, h % 2
                src, srs = po[0:64, 0:width], po[64:65, 0:width]
                rsum = rssb.tile([1, 1024], f32, name="rsum")
                nc.vector.tensor_copy(rsum[:, 0:width], srs)
                rinv = rssb.tile([1, 1024], f32r, name="rinv")
                with nc.allow_low_precision(reason="f32r rowsum reciprocal"):
                    nc.vector.reciprocal(rinv[:, 0:width], rsum[:, 0:width])
                bc = scps.tile([64, 1024], f32, name="bc", tag="sc")
                for half in range(width // 512):
                    nc.tensor.matmul(bc[:, 512 * half : 512 * (half + 1)],
                                     ones_sb[:],
                                     rinv[:, 512 * half : 512 * (half + 1)],
                                     start=True, stop=True)
                bc_sb = rssb.tile([64, 1024], f32, name="bc_sb")
                nc.scalar.copy(bc_sb[:, 0:width], bc[:, 0:width])
                nc.vector.tensor_tensor(
                    aoT_sb[rb][64 * hh : 64 * (hh + 1), tok0 : tok0 + width],
                    src, bc_sb[:, 0:width], op=OP.mult)

            def outproj_piece(krs, dt_):
                """one dout-tile of out-proj for token chunk krs (512 wide)."""
                yst = ysb.tile([128, 512], bf16, name="yst")
                yp = atps.tile([128, 512], f32, name="yp", tag="po")
                for ft in range(2):
                    nc.tensor.matmul(
                        yp[:], WT_sb[:, ft, 128 * dt_ : 128 * (dt_ + 1)],
                        aoT_sb[ft][:, 512 * krs : 512 * (krs + 1)],
                        start=(ft == 0), stop=(ft == 1))
                nc.vector.tensor_scalar_mul(yst[:], yp[:], s_out[:, dt_ : dt_ + 1])
                nc.sync.dma_start(
                    rs_in[krs][128 * dt_ : 128 * (dt_ + 1), :], yst[:])

            def outproj_finish(krs):
                nc.gpsimd.collective_compute(
                    "ReduceScatter", mybir.AluOpType.add,
                    replica_groups=[[0, 1, 2, 3], [4, 5, 6, 7]],
                    ins=[rs_in[krs].opt()], outs=[rs_out[krs].opt()])
                nc.gpsimd.dma_start(
                    out_ext[:, 512 * krs : 512 * (krs + 1)], rs_out[krs][:])  # cast

            def dummy_piece():
                """keep the PE activity monitor warm during the first block."""
                yp = scps.tile([128, 512], f32, name="ydum", tag="sc")
                nc.tensor.matmul(yp[:], xT_sb[:, 0, 0:128], whT_sb[:, 0, 0:512],
                                 start=True, stop=True)

            pieces = []
            # token blocks: exp/score width per block; later blocks smaller so
            # their ReduceScatter chunks overlap earlier compute
            blocks = [(0, 1024, (0, 1)), (1024, 512, (2,)), (1536, 512, (3,))]
            for tok0, width, krss in blocks:
                nhalf = width // 512
                j0 = NTT // 2 if tok0 == 0 else 0
                for h in range(HL):
                    rb, hh = h // 2, h % 2
                    po = atps.tile([128, 1024], f32, name="po")
                    prev_ex = None
                    for j in range(NTT):
                        sc = scps.tile([128, 1024], f32, name="sc", tag="sc")
                        for half in range(nhalf):
                            nc.tensor.matmul(
                                sc[:, 512 * half : 512 * (half + 1)],
                                kTd[h][:, 128 * j : 128 * (j + 1)],
                                qTd[h][:, tok0 + 512 * half :
                                       tok0 + 512 * (half + 1)],
                                start=True, stop=True)
                        ex = exsb.tile([128, 1024], bf16, name="ex")
                        nc.scalar.activation(ex[:, 0:width], sc[:, 0:width],
                                             AF.Exp, scale=b_all[:, j, h : h + 1])
                        # accumulate the PREVIOUS j so the next score matmuls
                        # sit ahead of exp-dependent work in the PE queue
                        if prev_ex is not None:
                            pj, pex = prev_ex
                            for half in range(nhalf):
                                nc.tensor.matmul(
                                    po[:, 512 * half : 512 * (half + 1)],
                                    v_sb[:, pj, 128 * h : 128 * (h + 1)],
                                    pex[:, 512 * half : 512 * (half + 1)],
                                    start=(pj == 0), stop=False)
                        prev_ex = (j, ex)
                        if pieces and j % 4 == 3:
                            pieces.pop(0)()
                    pj, pex = prev_ex
                    for half in range(nhalf):
                        nc.tensor.matmul(
                            po[:, 512 * half : 512 * (half + 1)],
                            v_sb[:, pj, 128 * h : 128 * (h + 1)],
                            pex[:, 512 * half : 512 * (half + 1)],
                            start=False, stop=True)
                    rs_dance(po, h, tok0, width)
                # queue this block's out-proj as filler for later blocks
                for krs in krss:
                    for dt_ in range(NDT):
                        pieces.append(
                            lambda krs=krs, dt_=dt_: outproj_piece(krs, dt_))
                    pieces.append(lambda krs=krs: outproj_finish(krs))
            for p in pieces:
                p()

    nc.compile()
    return nc


@functools.lru_cache(maxsize=1)
def _get_nc():
    return _build_bass()


def kernel(x: np.ndarray, w_qkv: np.ndarray, w_out: np.ndarray) -> np.ndarray:
    import ml_dtypes
    from concourse.bass_utils import run_bass_kernel_spmd

    x = np.asarray(x, dtype=np.float32)
    w_qkv = np.asarray(w_qkv, dtype=np.float32)
    w_out = np.asarray(w_out, dtype=np.float32)

    woutT = np.ascontiguousarray(w_out.T)
    in_maps = []
    for c in range(NCORES):
        b, g = c // GROUP, c % GROUP
        rows = np.concatenate([
            np.arange(DL * g, DL * (g + 1)),
            D + np.arange(DL * g, DL * (g + 1)),
            2 * D + np.arange(DL * g, DL * (g + 1)),
        ])
        in_maps.append({
            "xT": np.ascontiguousarray(x[b].T).astype(ml_dtypes.bfloat16),
            "wqkv": np.ascontiguousarray(w_qkv[rows]).astype(ml_dtypes.bfloat16),
            "woutT": np.ascontiguousarray(
                woutT[DL * g : DL * (g + 1)]).astype(ml_dtypes.bfloat16),
            "wout": w_out.astype(ml_dtypes.bfloat16),
        })

    nc = _get_nc()
    res = run_bass_kernel_spmd(nc, in_maps, core_ids=list(range(NCORES)))

    out = np.empty((B, T, D), dtype=np.float32)
    for c in range(NCORES):
        b, g = c // GROUP, c % GROUP
        out[b][:, DL * g : DL * (g + 1)] = res.results[c]["out"].astype(np.float32).T
    return out
